# revision 1
# baseline (speedup 1.0000x reference)
"""Trainium2 Bass kernel v2: causal spatial attention block, fp8-DoubleRow.

Data-parallel over batch across 8 NeuronCores (4 batches/core, no
collectives). PE plan per batch (cost = out-free-size x cyc/row; fp8
DoubleRow = 0.5 cyc/row and contracts two 128-deep k-tiles per instr):
  - x (with pos-emb channels pre-merged host-side) is DMA'd straight into
    an f32r-typed [128,2,S] tile; the fp8 copy for the Q/K path converts
    from it (DVE+Pool, one half each).
  - Q/K projections: fp8-DR, 16 matmuls of N=256 into [128,2,256] psum
    pairs; relu+bias+fp8-quantize in 8 tensor_scalar ops of 512 cols.
  - V projection: f32r (v-path needs >=bf16 accuracy; fp8 fails the 2e-2
    gate) + fp8-DR rank-1 bias (lhsT 0.25, rhs 4*bv).
  - vT stored SPLIT-fp8: hi = relu(psum) as fp8, lo = (psum max 0) - hi
    (one scalar_tensor_tensor). o-psum accumulates vt_hi@p + vt_lo@p,
    bf16-grade accuracy at fp8-DR speed.
  - scores: fp8-DR per (t-pair, 256-col sub-chunk); causal masking is
    ADDED on the PE (identity-lhsT DR matmuls of -240 tri/full tiles) so
    exp underflows to exact fp8 zeros.
  - p = exp(scores/16): ACT writes fp8 directly, one instr per pair tile.
  - o/dn accumulate region-wise into 512-wide acc psums (o0,o1,dn =
    3 banks; score/proj pairs are 1-bank tiles, bufs=5 -> 8 banks total).
  - denom reciprocal: DVE InstReciprocal (IEEE-exact); osb = o * rec.
  - outputs staged in one [128,2,S] f32 tile, DMA'd once per half per
    batch on the two HWDGE queues (SP + ACT) to dodge the Pool SWDGE tax.
"""
import numpy as np
from contextlib import ExitStack

import concourse.bass as bass
import concourse.mybir as mybir
import concourse.tile as tile
from concourse import bacc
from concourse.bass_utils import run_bass_kernel_spmd

F32 = mybir.dt.float32
F32R = mybir.dt.float32r
BF16 = mybir.dt.bfloat16
FP8 = mybir.dt.float8e4
AF = mybir.ActivationFunctionType
ALU = mybir.AluOpType
DR = mybir.MatmulPerfMode.DoubleRow

B, C, L, EMB = 32, 224, 32, 16
S = L * L            # 1024
CIN = 256
NCORES = 8
NB = B // NCORES     # 4 batches per core
MASKV = -240.0       # fp8e4 max finite; exp((score-240)/16) -> fp8 zero


def _pos_embeddings() -> np.ndarray:
    pos = np.arange(L)[:, None].astype(np.float64)
    j = np.arange(EMB)[None, :]
    enc = pos / np.power(10000.0, 2.0 * (j // 2) / EMB)
    enc[0, :] = 0.0
    enc[1:, 0::2] = np.sin(enc[1:, 0::2])
    enc[1:, 1::2] = np.cos(enc[1:, 1::2])
    t = enc.astype(np.float32)
    x = np.tile(t.reshape(1, EMB, L, 1), (1, 1, 1, L))
    y = np.tile(t.reshape(1, EMB, 1, L), (1, 1, L, 1))
    pe = np.concatenate((x, y), axis=1)[0]
    return np.ascontiguousarray(pe.reshape(2 * EMB, S))


def _pin_act_tables():
    from concourse import bacc as _bacc
    real = _bacc.get_activation_tables
    def patched(arch):
        tables = real(arch)
        keep = "natural_log_exp_and_others"
        assert keep in tables
        return {name: (funcs if name == keep else set())
                for name, funcs in tables.items()}
    _bacc.get_activation_tables = patched
    return real


def build(reps: int = 1):
    real_tables = _pin_act_tables()
    nc = bacc.Bacc("TRN2", target_bir_lowering=False, debug=False,
                   num_devices=NCORES)
    # x has pe channels pre-merged host-side: [NB, 256, S], consumed as f32r
    x_d = nc.declare_dram_parameter("xm", [NB, 2, 128, S], F32R, isOutput=False)
    wq_d = nc.declare_dram_parameter("wqt", [CIN, 256], F32, isOutput=False)
    wk_d = nc.declare_dram_parameter("wkt", [CIN, 256], F32, isOutput=False)
    wv_d = nc.declare_dram_parameter("wvt", [CIN, 256], F32R, isOutput=False)
    bqk_d = nc.declare_dram_parameter("bqk", [128, 4], F32, isOutput=False)
    bv_d = nc.declare_dram_parameter("bv", [256], F32, isOutput=False)
    # mconst: [identity | strict-lower-tri * -240]
    mk_d = nc.declare_dram_parameter("mconst", [128, 256], F32, isOutput=False)
    out_d = nc.declare_dram_parameter("out", [NB, 256, S], F32, isOutput=True)

    with ExitStack() as ctx:
        tc = ctx.enter_context(tile.TileContext(nc))
        const = ctx.enter_context(tc.tile_pool(name="const", bufs=1))
        xrp = ctx.enter_context(tc.tile_pool(name="xr", bufs=2))
        x8p = ctx.enter_context(tc.tile_pool(name="x8", bufs=2))
        qkp = ctx.enter_context(tc.tile_pool(name="qk", bufs=2))
        vtp = ctx.enter_context(tc.tile_pool(name="vt", bufs=2))
        pp = ctx.enter_context(tc.tile_pool(name="p", bufs=12))
        ep = ctx.enter_context(tc.tile_pool(name="epi", bufs=3))
        op = ctx.enter_context(tc.tile_pool(name="ob", bufs=2))
        ps_p = ctx.enter_context(tc.tile_pool(name="pair", bufs=5, space="PSUM"))
        ps_acc = ctx.enter_context(tc.tile_pool(name="acc", bufs=3, space="PSUM"))

        consts = {}

        def _load_wv():
            wvr = const.tile([128, 2, 256], F32R, tag="wv_r")
            nc.sync.dma_start(wvr[:], wv_d[:].rearrange("(ci p) m -> p ci m", p=128))
            consts["wv"] = wvr
            b1 = const.tile([1, 2, 128], FP8, tag="b1")
            nc.vector.memset(b1[:], 0.0)
            nc.vector.memset(b1[0:1, 0, :], 0.25)
            consts["b1"] = b1
            bv8 = const.tile([1, 2, 256], FP8, tag="bv8")
            nc.vector.memset(bv8[:], 0.0)
            # SWDGE casting DMA: f32 DRAM -> fp8 SBUF, bit-exact RNE
            # (host ships bv pre-scaled by 4)
            nc.gpsimd.dma_start(bv8[0:1, 0, :], bv_d[:].unsqueeze(0))
            consts["bv8"] = bv8


        def _make_ones_and_warm():
            ones8 = const.tile([128, 2, 128], FP8, tag="ones8")
            nc.vector.memset(ones8[:], 1.0)
            consts["ones8"] = ones8
            # dummy activation: pulls the ~1.3us LoadActFuncSet off the
            # batch-0 critical chain into the startup DMA window
            wa = const.tile([1, 128], F32, tag="warm_act")
            nc.scalar.activation(wa[:], ones8[0:1, 0, :], AF.Exp)
            warm = ps_acc.tile([128, 512], F32, tag="acc")
            for _ in range(8):
                nc.tensor.matmul(warm[:, 0:128], ones8[:, 0, :], ones8[:, 0, :],
                                 start=True, stop=True)

        def _load_weights():
            # SWDGE casting DMAs (f32 DRAM -> fp8 SBUF, RNE): no staging
            # tiles, no DVE/Pool conversion passes, Pool is idle at startup
            for nm, dram in (("wq", wq_d), ("wk", wk_d)):
                w8 = const.tile([128, 2, 256], FP8, tag=nm + "_8")
                nc.gpsimd.dma_start(
                    w8[:], dram[:].rearrange("(ci p) m -> p ci m", p=128))
                consts[nm] = w8
            bqk = const.tile([128, 4], F32, tag="bqk")
            nc.scalar.dma_start(bqk[:], bqk_d[:])
            consts["bqk"] = bqk
            i8 = const.tile([128, 2, 128], FP8, tag="i8")
            nc.gpsimd.memset(i8[:], 0.0)
            nc.gpsimd.dma_start(i8[:, 0, :], mk_d[:, 0:128])
            consts["i8"] = i8
            m8 = const.tile([128, 2, 128], FP8, tag="m8")
            nc.gpsimd.memset(m8[:], 0.0)
            nc.gpsimd.dma_start(m8[:, 0, :], mk_d[:, 128:256])
            consts["m8"] = m8
            fm8 = const.tile([128, 2, 256], FP8, tag="fm8")
            nc.gpsimd.memset(fm8[:], 0.0)
            nc.gpsimd.memset(fm8[:, 0, 0:128], MASKV)
            nc.gpsimd.dma_start(fm8[:, 0, 128:256], mk_d[:, 128:256])
            consts["fm8"] = fm8

        def load_x0(b):
            first = "wv" not in consts
            x0r = xrp.tile([128, 2, S], F32R, tag="x0r")
            x08 = x8p.tile([128, 2, S], FP8, tag="x08")
            for hi, (h0, h1) in enumerate(((0, 512), (512, S))):
                nc.sync.dma_start(
                    x0r[:, :, h0:h1],
                    x_d[b, :, :, h0:h1].rearrange("sl p s -> p sl s"))
                if first and hi == 0:
                    # batch 0: x half-0 heads the SP queue (the qk/scores
                    # chain gates startup); wv can land after it since the
                    # V path is consumed via the deferred o/dn queue
                    _load_wv()
                    _make_ones_and_warm()
                # fp8 copy on Pool: SBUF->SBUF is the only op class the
                # Pool engine supports in this toolchain (no PSUM reads,
                # no scalar_tensor_tensor), so give it all of this one
                nc.gpsimd.tensor_copy(x08[:, :, h0:h1], x0r[:, :, h0:h1])
            if "wq" not in consts:
                _load_weights()
            return x08, x0r

        x_next = load_x0(0)

        # o/dn jobs are deferred in a queue that DRAINS ACROSS BATCH
        # BOUNDARIES: batch b's tail o/dn + epilogue interleave with batch
        # b+1's projections instead of bunching at the boundary. Jobs carry
        # all batch-local tiles explicitly (no closure capture).
        pending = []

        def attn_odn(job):
            (pt, a, sub, o0, o1, dn, first, last, ck0, vth, vtl,
             osb_all, b, last_batch) = job
            reg = slice(sub * 256 - ck0, sub * 256 - ck0 + 256)
            nc.tensor.matmul(dn[:, reg], consts["ones8"][:], pt[:],
                             start=first, stop=last, perf_mode=DR)
            nc.tensor.matmul(o0[:, reg], vth[:, 2 * a:2 * a + 2, 0:128],
                             pt[:], start=first, stop=False, perf_mode=DR)
            nc.tensor.matmul(o0[:, reg], vtl[:, 2 * a:2 * a + 2, 0:128],
                             pt[:], start=False, stop=last, perf_mode=DR)
            nc.tensor.matmul(o1[:, reg], vth[:, 2 * a:2 * a + 2, 128:256],
                             pt[:], start=first, stop=False, perf_mode=DR)
            nc.tensor.matmul(o1[:, reg], vtl[:, 2 * a:2 * a + 2, 128:256],
                             pt[:], start=False, stop=last, perf_mode=DR)
            if not (last and sub % 2 == 1):
                return
            # final o/dn of a 512-chunk: normalize on DVE while PE continues
            rec = ep.tile([128, 512], F32, tag="rec")
            nc.vector.reciprocal(rec[:], dn[:])
            for m, om in enumerate((o0, o1)):
                nc.vector.tensor_tensor(osb_all[:, m, ck0:ck0 + 512],
                                        om[:], rec[:], op=ALU.mult)
            if sub == 3:
                # whole batch normalized: ship it
                if last_batch:
                    for m in range(2):
                        nc.sync.dma_start(
                            out_d[b, m * 128:(m + 1) * 128, 0:512],
                            osb_all[:, m, 0:512])
                        nc.scalar.dma_start(
                            out_d[b, m * 128:(m + 1) * 128, 512:],
                            osb_all[:, m, 512:])
                else:
                    nc.sync.dma_start(out_d[b, 0:128, :], osb_all[:, 0, :])
                    nc.sync.dma_start(out_d[b, 128:256, :], osb_all[:, 1, :])

        def attn_push(job):
            pending.append(job)
            limit = 3 if job[-1] else 10
            if len(pending) > limit:
                attn_odn(pending.pop(0))

        def attn_flush():
            while pending:
                attn_odn(pending.pop(0))

        seq = [b for _ in range(reps) for b in range(NB)]
        for bi, b in enumerate(seq):
            is_last = (bi == len(seq) - 1)
            x08, x0r = x_next

            wq8, wk8, wvr = consts["wq"], consts["wk"], consts["wv"]
            bqk, b1, bv8 = consts["bqk"], consts["b1"], consts["bv8"]
            ones8, i8, m8, fm8 = (consts["ones8"], consts["i8"],
                                  consts["m8"], consts["fm8"])

            # ---- V projection -> split-fp8 vT (hi + lo), ti-pairs ----
            vth = vtp.tile([128, 8, 256], FP8, tag="vth")
            vtl = vtp.tile([128, 8, 256], FP8, tag="vtl")
            rv = vtp.tile([128, 8, 256], BF16, tag="rv")
            vt_done = [0]

            def ensure_vt(npairs):
                for a in range(vt_done[0], npairs):
                    vp = ps_p.tile([128, 2, 256], F32, tag="pair")
                    for sl in range(2):
                        ti = 2 * a + sl
                        ts = slice(ti * 128, (ti + 1) * 128)
                        nc.tensor.matmul(vp[:, sl, :], x0r[:, 0, ts],
                                         wvr[:, 0, :], start=True, stop=False)
                        nc.tensor.matmul(vp[:, sl, :], x0r[:, 1, ts],
                                         wvr[:, 1, :], start=False, stop=False)
                        nc.tensor.matmul(vp[:, sl, :], b1[:], bv8[:],
                                         start=False, stop=True, perf_mode=DR)
                    # drain relu(v) ONCE to bf16 SBUF (ACT/DVE are the
                    # psum-drain bottleneck pair); the fp8 hi/lo split is
                    # then pure SBUF work on the otherwise-idle Pool
                    dst_r = rv[:, 2 * a:2 * a + 2, :]
                    dst_h = vth[:, 2 * a:2 * a + 2, :]
                    dst_l = vtl[:, 2 * a:2 * a + 2, :]
                    if a < 3:
                        nc.scalar.activation(dst_r, vp[:], AF.Relu)
                    else:
                        nc.vector.tensor_scalar(dst_r, vp[:], 0.0, None,
                                                op0=ALU.max)
                    nc.gpsimd.tensor_copy(dst_h, dst_r)
                    nc.gpsimd.tensor_tensor(dst_l, dst_r, dst_h,
                                            op=ALU.subtract)
                vt_done[0] = max(vt_done[0], npairs)

            # ---- Q/K projections for one s-half (fp8 DoubleRow) ----
            q8t = qkp.tile([128, 2, S], FP8, tag="q8")
            k8t = qkp.tile([128, 2, S], FP8, tag="k8")

            def qk_proj(sj):
                ss = slice(sj * 512, (sj + 1) * 512)
                for pi, (w8, dst) in enumerate(((wq8, q8t), (wk8, k8t))):
                    for m in range(2):
                        qp = ps_p.tile([128, 2, 256], F32, tag="pair")
                        for sq in range(2):
                            s0 = sj * 512 + sq * 256
                            nc.tensor.matmul(
                                qp[:, sq, :],
                                w8[:, :, m * 128:(m + 1) * 128],
                                x08[:, :, s0:s0 + 256],
                                start=True, stop=True, perf_mode=DR)
                        bias_ap = bqk[:, 2 * pi + m:2 * pi + m + 1]
                        dst_ap = dst[:, m, ss].rearrange("p (a q) -> p a q", a=2)
                        if 2 * pi + m == 0 and sj == 0:
                            nc.scalar.activation(dst_ap, qp[:], AF.Relu,
                                                 bias=bias_ap)
                        else:
                            nc.vector.tensor_scalar(dst_ap, qp[:], bias_ap,
                                                    0.0, op0=ALU.add,
                                                    op1=ALU.max)

            # ---- attention sub-chunk: t-pair a, s columns [sc0, sc0+256) ----
            osb_all = op.tile([128, 2, S], F32, tag="osb")

            def attn_scores(a, sub):
                sc0 = sub * 256
                diag = (a == sub)
                sp = ps_p.tile([128, 2, 256], F32, tag="pair")
                for sl in range(2):
                    ti = 2 * a + sl
                    nc.tensor.matmul(sp[:, sl, :],
                                     k8t[:, :, ti * 128:(ti + 1) * 128],
                                     q8t[:, :, sc0:sc0 + 256],
                                     start=True, stop=not diag, perf_mode=DR)
                    if diag:
                        # close this slot's group before slot1 opens its own
                        # (one pending group per psum zero region)
                        mask_rhs = m8 if sl == 0 else fm8
                        mask_out = sp[:, 0, 0:128] if sl == 0 else sp[:, 1, :]
                        nc.tensor.matmul(mask_out, i8[:], mask_rhs[:],
                                         start=False, stop=True, perf_mode=DR)
                pt = pp.tile([128, 2, 256], FP8, tag="pt")
                nc.scalar.activation(pt[:], sp[:], AF.Exp, scale=0.0625)
                return pt

            # ---- batch schedule ----
            # qk first: the scores path (qk matmul -> relu -> scores -> exp)
            # is the longest cross-engine chain; vt is only needed 2
            # deferred iterations later
            qk_proj(0)
            ensure_vt(2)
            for half in range(2):
                if half == 1:
                    qk_proj(1)
                    ensure_vt(4)
                ck0 = half * 512
                o0 = ps_acc.tile([128, 512], F32, tag="acc")
                o1 = ps_acc.tile([128, 512], F32, tag="acc")
                dn = ps_acc.tile([128, 512], F32, tag="acc")
                for sub in (2 * half, 2 * half + 1):
                    if sub == 3 and not is_last:
                        # prefetch + fp8-convert next batch now: the copies
                        # finish during this batch's tail, so the next
                        # batch's qk matmuls start without waiting
                        x_next = load_x0(seq[bi + 1])
                    for a in range(sub + 1):
                        pt = attn_scores(a, sub)
                        attn_push((pt, a, sub, o0, o1, dn,
                                   a == 0, a == sub, ck0, vth, vtl,
                                   osb_all, b, is_last))
            if is_last:
                attn_flush()

    try:
        nc.finalize()
    finally:
        from concourse import bacc as _bacc
        _bacc.get_activation_tables = real_tables
    return nc


def make_in_maps(x, wq, bq, wk, bk, wv, bv):
    x_r = x.reshape(B, C, S).astype(np.float32)
    pe = _pos_embeddings()
    xm = np.concatenate(
        [x_r, np.broadcast_to(pe[None], (B, 2 * EMB, S))], axis=1)
    xm = np.ascontiguousarray(xm.reshape(B, 2, 128, S))
    wqt = np.ascontiguousarray(wq.T.astype(np.float32))
    wkt = np.ascontiguousarray(wk.T.astype(np.float32))
    wvt = np.ascontiguousarray(wv.T.astype(np.float32))
    bq = bq.astype(np.float32)
    bk = bk.astype(np.float32)
    bqk = np.ascontiguousarray(
        np.stack([bq[:128], bq[128:], bk[:128], bk[128:]], axis=1))
    ident = np.eye(128, dtype=np.float32)
    smask = np.tril(np.ones((128, 128), np.float32), k=-1) * MASKV
    mconst = np.ascontiguousarray(np.concatenate([ident, smask], axis=1))
    common = dict(wqt=wqt, wkt=wkt, wvt=wvt, bqk=bqk,
                  bv=np.ascontiguousarray(4.0 * bv.astype(np.float32)),
                  mconst=mconst)
    return [dict(xm=np.ascontiguousarray(xm[i * NB:(i + 1) * NB]), **common)
            for i in range(NCORES)]


_NC_CACHE = None


def kernel(x, wq, bq, wk, bk, wv, bv):
    global _NC_CACHE
    if _NC_CACHE is None:
        _NC_CACHE = build()
    nc = _NC_CACHE
    in_maps = make_in_maps(x, wq, bq, wk, bk, wv, bv)
    res = run_bass_kernel_spmd(nc, in_maps, core_ids=list(range(NCORES)))
    out = np.concatenate([res.results[i]["out"] for i in range(NCORES)], axis=0)
    return np.ascontiguousarray(out.reshape(B, 256, L, L).astype(np.float32))



# revision 14
# speedup vs baseline: 2.9194x; 2.9194x over previous
"""Trainium2 Bass kernel v3: causal spatial attention block, all-f32,
minimal-instruction design.

Data-parallel over batch across 8 NeuronCores (4 batches/core, no
collectives). The execution backend charges a large, mostly
size-independent cost per *instruction* (ACT worst, ~250us; MM ~30-50us;
DVE ~30us), so this kernel minimizes instruction count and avoids narrow
dtypes entirely (fp8/bf16 conversions are pure overhead there):
  - Q/K projections: 8 giant matmuls of N=4096 (all 4 batches at once)
    into full-PSUM [128,4096] tiles; one fused bias+relu drain per
    (proj, chan-half) -> 4 DVE instructions.
  - V projection per batch: 16 matmuls produce vT directly in [t, v]
    layout (lhsT = x-tile, rhs = wv), 2 rank-1 bias matmuls, one relu
    drain.
  - scores per batch: 2 phases x 4 t-tiles x (2 k-matmuls N=1024 + 1
    additive-mask matmul via identity-lhsT x shifted-window of one
    [128,1920] mask pattern); exp(score/16) applied by ONE activation
    per phase over the whole [128,4,1024] PSUM; masked entries underflow
    to exact f32 zeros, so o/dn can contract full 1024-wide tiles.
  - o/dn per batch: 24 matmuls (8 t-tiles x {v-half0, v-half1, ones});
    reciprocal + 2 multiplies normalize during the PSUM drain.
All PSUM phases share one [128,4096] tile (8 banks), start=True only on
the first matmul touching each 512-col bank.
"""
import numpy as np
from contextlib import ExitStack

import concourse.bass as bass
import concourse.mybir as mybir
import concourse.tile as tile
from concourse import bacc
from concourse.bass_utils import run_bass_kernel_spmd

F32 = mybir.dt.float32
AF = mybir.ActivationFunctionType
ALU = mybir.AluOpType

B, C, L, EMB = 32, 224, 32, 16
S = L * L            # 1024
CIN = 256
NCORES = 8
NB = B // NCORES     # 4 batches per core
SB = NB * S          # 4096 columns: all 4 batches side by side
MASKV = -30000.0     # exp((x-30000)/16) underflows to exact f32 zero


def _pos_embeddings() -> np.ndarray:
    pos = np.arange(L)[:, None].astype(np.float64)
    j = np.arange(EMB)[None, :]
    enc = pos / np.power(10000.0, 2.0 * (j // 2) / EMB)
    enc[0, :] = 0.0
    enc[1:, 0::2] = np.sin(enc[1:, 0::2])
    enc[1:, 1::2] = np.cos(enc[1:, 1::2])
    t = enc.astype(np.float32)
    x = np.tile(t.reshape(1, EMB, L, 1), (1, 1, 1, L))
    y = np.tile(t.reshape(1, EMB, 1, L), (1, 1, L, 1))
    pe = np.concatenate((x, y), axis=1)[0]
    return np.ascontiguousarray(pe.reshape(2 * EMB, S))


def build(reps: int = 1):
    nc = bacc.Bacc("TRN2", target_bir_lowering=False, debug=False,
                   num_devices=NCORES)
    x_d = nc.declare_dram_parameter("xm", [2, 128, NB, S], F32, isOutput=False)
    wq_d = nc.declare_dram_parameter("wqt", [CIN, 256], F32, isOutput=False)
    wk_d = nc.declare_dram_parameter("wkt", [CIN, 256], F32, isOutput=False)
    wv_d = nc.declare_dram_parameter("wvt", [CIN, 256], F32, isOutput=False)
    bqk_d = nc.declare_dram_parameter("bqk", [128, 4], F32, isOutput=False)
    bv_d = nc.declare_dram_parameter("bvr", [1, 512], F32, isOutput=False)
    # mconst: [ identity(128) | E-mask(1920) ]  (E[p][j] = MASKV if j < 896+p)
    mk_d = nc.declare_dram_parameter("mconst", [128, 2048], F32, isOutput=False)
    out_d = nc.declare_dram_parameter("out", [NB, 256, S], F32, isOutput=True)

    with ExitStack() as ctx:
        tc = ctx.enter_context(tile.TileContext(nc))
        const = ctx.enter_context(tc.tile_pool(name="const", bufs=1))
        xp = ctx.enter_context(tc.tile_pool(name="xp", bufs=1))
        qkp = ctx.enter_context(tc.tile_pool(name="qk", bufs=1))
        vtp = ctx.enter_context(tc.tile_pool(name="vt", bufs=2))
        ppool = ctx.enter_context(tc.tile_pool(name="pp", bufs=1))
        op = ctx.enter_context(tc.tile_pool(name="ob", bufs=2))
        rp = ctx.enter_context(tc.tile_pool(name="rp", bufs=2))
        psp = ctx.enter_context(tc.tile_pool(name="ps", bufs=1, space="PSUM"))

        # ---- constants ----
        wqt = const.tile([128, 2, 256], F32, tag="wqt")
        nc.sync.dma_start(wqt[:], wq_d[:].rearrange("(ci p) m -> p ci m", p=128))
        wkt = const.tile([128, 2, 256], F32, tag="wkt")
        nc.sync.dma_start(wkt[:], wk_d[:].rearrange("(ci p) m -> p ci m", p=128))
        wvt = const.tile([128, 2, 256], F32, tag="wvt")
        nc.sync.dma_start(wvt[:], wv_d[:].rearrange("(ci p) m -> p ci m", p=128))
        bqk = const.tile([128, 4], F32, tag="bqk")
        nc.sync.dma_start(bqk[:], bqk_d[:])
        bvr = const.tile([1, 512], F32, tag="bvr")
        nc.sync.dma_start(bvr[:], bv_d[:])
        mck = const.tile([128, 2048], F32, tag="mck")
        nc.sync.dma_start(mck[:], mk_d[:])
        ones1 = const.tile([1, 128], F32, tag="ones1")
        nc.vector.memset(ones1[:], 1.0)
        ones128 = const.tile([128, 128], F32, tag="ones128")
        nc.vector.memset(ones128[:], 1.0)
        ident = mck[:, 0:128]
        emask = mck[:, 128:2048]           # [128, 1920]

        ps = psp.tile([128, 4096], F32, tag="ps")

        for rep in range(reps):
            # ---- x load: all 4 batches -> [128, 2(ci), 4096] ----
            xt = xp.tile([128, 2, SB], F32, tag="xt")
            nc.sync.dma_start(xt[:], x_d[:].rearrange("ci p b s -> p ci (b s)"))

            # ---- Q/K projections, all batches at once ----
            q = qkp.tile([128, 2, SB], F32, tag="q")
            k = qkp.tile([128, 2, SB], F32, tag="k")
            for pi, (wt, dst) in enumerate(((wqt, q), (wkt, k))):
                for m in range(2):
                    for c in range(8):
                        cs = slice(512 * c, 512 * c + 512)
                        nc.tensor.matmul(ps[:, cs],
                                         wt[:, 0, 128 * m:128 * m + 128],
                                         xt[:, 0, cs], start=True, stop=False)
                        nc.tensor.matmul(ps[:, cs],
                                         wt[:, 1, 128 * m:128 * m + 128],
                                         xt[:, 1, cs], start=False, stop=True)
                    nc.vector.tensor_scalar(dst[:, m, :], ps[:],
                                            bqk[:, 2 * pi + m:2 * pi + m + 1],
                                            0.0, op0=ALU.add, op1=ALU.max)

            for b in range(NB):
                s0 = b * S

                # ---- V projection -> vT [128(t), 8, 256(v)] ----
                vt = vtp.tile([128, 8, 256], F32, tag="vt")
                for a in range(8):
                    for ci in range(2):
                        nc.tensor.matmul(
                            ps[:, 256 * a:256 * a + 256],
                            xt[:, ci, s0 + 128 * a:s0 + 128 * a + 128],
                            wvt[:, ci, :],
                            start=(a % 2 == 0 and ci == 0), stop=False)
                for h in range(4):
                    nc.tensor.matmul(ps[:, 512 * h:512 * h + 512],
                                     ones1[:], bvr[:],
                                     start=False, stop=True)
                nc.vector.tensor_scalar(
                    vt[:], ps[:, 0:2048].rearrange("p (a v) -> p a v", a=8),
                    0.0, None, op0=ALU.max)

                # ---- scores + exp: 2 phases of 4 t-tiles ----
                pt = ppool.tile([128, 8, S], F32, tag="pt")
                for ph in range(2):
                    for j in range(4):
                        i = 4 * ph + j
                        tc_ = slice(s0 + 128 * i, 128 * i + s0 + 128)
                        for c in range(2):
                            sc = ps[:, 1024 * j + 512 * c:
                                    1024 * j + 512 * c + 512]
                            j0 = 896 - 128 * i + 512 * c
                            fully_masked = j0 <= 384
                            has_mask = j0 < 1024
                            if not fully_masked:
                                qs = slice(s0 + 512 * c, s0 + 512 * c + 512)
                                nc.tensor.matmul(sc, k[:, 0, tc_],
                                                 q[:, 0, qs], start=True,
                                                 stop=not has_mask)
                                nc.tensor.matmul(sc, k[:, 1, tc_],
                                                 q[:, 1, qs], start=False,
                                                 stop=not has_mask)
                            if has_mask:
                                nc.tensor.matmul(sc, ident,
                                                 emask[:, j0:j0 + 512],
                                                 start=fully_masked, stop=True)
                    nc.scalar.activation(
                        pt[:, 4 * ph:4 * ph + 4, :],
                        ps[:].rearrange("p (a s) -> p a s", a=4),
                        AF.Exp, scale=0.0625)

                # ---- o / dn ----
                om0 = ps[:, 0:1024]
                om1 = ps[:, 1024:2048]
                dnp = ps[:, 2048:3072]
                for c in range(2):
                    ntile = 4 * c + 4   # causal: chunk c only sees t < 512(c+1)
                    for i in range(ntile):
                        st, sp_ = (i == 0), (i == ntile - 1)
                        pcs = pt[:, i, 512 * c:512 * c + 512]
                        for roff, lh in ((0, vt[:, i, 0:128]),
                                         (1024, vt[:, i, 128:256]),
                                         (2048, ones128[:])):
                            nc.tensor.matmul(
                                ps[:, roff + 512 * c:roff + 512 * c + 512],
                                lh, pcs, start=st, stop=sp_)

                # ---- normalize + store ----
                rec = rp.tile([128, S], F32, tag="rec")
                nc.vector.reciprocal(rec[:], dnp)
                osb = op.tile([128, 2, S], F32, tag="osb")
                nc.vector.tensor_tensor(osb[:, 0, :], om0, rec[:], op=ALU.mult)
                nc.vector.tensor_tensor(osb[:, 1, :], om1, rec[:], op=ALU.mult)
                nc.sync.dma_start(
                    out_d[b][:].rearrange("(m p) s -> p m s", p=128), osb[:])

    nc.finalize()
    return nc


def make_in_maps(x, wq, bq, wk, bk, wv, bv):
    x_r = x.reshape(B, C, S).astype(np.float32)
    pe = _pos_embeddings()
    xm = np.concatenate(
        [x_r, np.broadcast_to(pe[None], (B, 2 * EMB, S))], axis=1)
    # [B, 2, 128, S] -> per-core [2, 128, NB, S] so the on-device DMA can
    # group (b s) into adjacent columns
    xm = xm.reshape(B, 2, 128, S)
    wqt = np.ascontiguousarray(wq.T.astype(np.float32))
    wkt = np.ascontiguousarray(wk.T.astype(np.float32))
    wvt = np.ascontiguousarray(wv.T.astype(np.float32))
    bq = bq.astype(np.float32)
    bk = bk.astype(np.float32)
    bqk = np.ascontiguousarray(
        np.stack([bq[:128], bq[128:], bk[:128], bk[128:]], axis=1))
    bvr = np.ascontiguousarray(np.tile(bv.astype(np.float32), 2)[None, :])
    ident = np.eye(128, dtype=np.float32)
    jj = np.arange(1920)[None, :]
    ppn = np.arange(128)[:, None]
    emask = np.where(jj < 896 + ppn, np.float32(MASKV), np.float32(0.0))
    mconst = np.ascontiguousarray(
        np.concatenate([ident, emask.astype(np.float32)], axis=1))
    common = dict(wqt=wqt, wkt=wkt, wvt=wvt, bqk=bqk, bvr=bvr, mconst=mconst)
    return [dict(xm=np.ascontiguousarray(
                     xm[i * NB:(i + 1) * NB].transpose(1, 2, 0, 3)), **common)
            for i in range(NCORES)]


_NC_CACHE = None


def kernel(x, wq, bq, wk, bk, wv, bv):
    global _NC_CACHE
    if _NC_CACHE is None:
        _NC_CACHE = build()
    nc = _NC_CACHE
    in_maps = make_in_maps(x, wq, bq, wk, bk, wv, bv)
    res = run_bass_kernel_spmd(nc, in_maps, core_ids=list(range(NCORES)))
    out = np.concatenate([res.results[i]["out"] for i in range(NCORES)], axis=0)
    return np.ascontiguousarray(out.reshape(B, 256, L, L).astype(np.float32))


# revision 18
# speedup vs baseline: 3.5632x; 1.2205x over previous
"""Trainium2 Bass kernel v3: causal spatial attention block, all-f32,
minimal-instruction design.

Data-parallel over batch across 8 NeuronCores (4 batches/core, no
collectives). The execution backend charges a large, mostly
size-independent cost per *instruction* (ACT worst, ~250us; MM ~30-50us;
DVE ~30us), so this kernel minimizes instruction count and avoids narrow
dtypes entirely (fp8/bf16 conversions are pure overhead there):
  - Q/K projections: 8 giant matmuls of N=4096 (all 4 batches at once)
    into full-PSUM [128,4096] tiles; one fused bias+relu drain per
    (proj, chan-half) -> 4 DVE instructions.
  - V projection per batch: 16 matmuls produce vT directly in [t, v]
    layout (lhsT = x-tile, rhs = wv), 2 rank-1 bias matmuls, one relu
    drain.
  - scores per batch: 2 phases x 4 t-tiles x (2 k-matmuls N=1024 + 1
    additive-mask matmul via identity-lhsT x shifted-window of one
    [128,1920] mask pattern); exp(score/16) applied by ONE activation
    per phase over the whole [128,4,1024] PSUM; masked entries underflow
    to exact f32 zeros, so o/dn can contract full 1024-wide tiles.
  - o/dn per batch: 24 matmuls (8 t-tiles x {v-half0, v-half1, ones});
    reciprocal + 2 multiplies normalize during the PSUM drain.
All PSUM phases share one [128,4096] tile (8 banks), start=True only on
the first matmul touching each 512-col bank.
"""
import numpy as np
from contextlib import ExitStack

import concourse.bass as bass
import concourse.mybir as mybir
import concourse.tile as tile
from concourse import bacc
from concourse.bass_utils import run_bass_kernel_spmd

F32 = mybir.dt.float32
FP8 = mybir.dt.float8e4
AF = mybir.ActivationFunctionType
ALU = mybir.AluOpType
DR = mybir.MatmulPerfMode.DoubleRow

B, C, L, EMB = 32, 224, 32, 16
S = L * L            # 1024
CIN = 256
NCORES = 8
NB = B // NCORES     # 4 batches per core
SB = NB * S          # 4096 columns: all 4 batches side by side
MASKV = -30000.0     # exp((x-30000)/16) underflows to exact f32 zero


def _pos_embeddings() -> np.ndarray:
    pos = np.arange(L)[:, None].astype(np.float64)
    j = np.arange(EMB)[None, :]
    enc = pos / np.power(10000.0, 2.0 * (j // 2) / EMB)
    enc[0, :] = 0.0
    enc[1:, 0::2] = np.sin(enc[1:, 0::2])
    enc[1:, 1::2] = np.cos(enc[1:, 1::2])
    t = enc.astype(np.float32)
    x = np.tile(t.reshape(1, EMB, L, 1), (1, 1, 1, L))
    y = np.tile(t.reshape(1, EMB, 1, L), (1, 1, L, 1))
    pe = np.concatenate((x, y), axis=1)[0]
    return np.ascontiguousarray(pe.reshape(2 * EMB, S))


def build(reps: int = 1):
    nc = bacc.Bacc("TRN2", target_bir_lowering=False, debug=False,
                   num_devices=NCORES)
    x_d = nc.declare_dram_parameter("xm", [2, 128, NB, S], F32, isOutput=False)
    wq_d = nc.declare_dram_parameter("wqt", [CIN, 256], F32, isOutput=False)
    wk_d = nc.declare_dram_parameter("wkt", [CIN, 256], F32, isOutput=False)
    wv_d = nc.declare_dram_parameter("wvt", [CIN, 256], F32, isOutput=False)
    bqk_d = nc.declare_dram_parameter("bqk", [128, 4], F32, isOutput=False)
    bv_d = nc.declare_dram_parameter("bvr", [1, 512], F32, isOutput=False)
    # mconst: [ identity(128) | E-mask(1920) ]  (E[p][j] = MASKV if j < 896+p)
    mk_d = nc.declare_dram_parameter("mconst", [128, 2048], F32, isOutput=False)
    out_d = nc.declare_dram_parameter("out", [NB, 256, S], F32, isOutput=True)

    with ExitStack() as ctx:
        tc = ctx.enter_context(tile.TileContext(nc))
        const = ctx.enter_context(tc.tile_pool(name="const", bufs=1))
        xp = ctx.enter_context(tc.tile_pool(name="xp", bufs=1))
        qkp = ctx.enter_context(tc.tile_pool(name="qk", bufs=1))
        vtp = ctx.enter_context(tc.tile_pool(name="vt", bufs=2))
        ppool = ctx.enter_context(tc.tile_pool(name="pp", bufs=1))
        op = ctx.enter_context(tc.tile_pool(name="ob", bufs=2))
        rp = ctx.enter_context(tc.tile_pool(name="rp", bufs=2))
        psp = ctx.enter_context(tc.tile_pool(name="ps", bufs=1, space="PSUM"))

        # ---- constants ----
        # SWDGE casting DMAs: f32 DRAM -> fp8 SBUF, bit-exact RNE
        wqt = const.tile([128, 2, 256], FP8, tag="wqt")
        nc.gpsimd.dma_start(wqt[:],
                            wq_d[:].rearrange("(ci p) m -> p ci m", p=128))
        wkt = const.tile([128, 2, 256], FP8, tag="wkt")
        nc.gpsimd.dma_start(wkt[:],
                            wk_d[:].rearrange("(ci p) m -> p ci m", p=128))
        wvt = const.tile([128, 2, 256], F32, tag="wvt")
        nc.sync.dma_start(wvt[:], wv_d[:].rearrange("(ci p) m -> p ci m", p=128))
        bqk = const.tile([128, 4], F32, tag="bqk")
        nc.sync.dma_start(bqk[:], bqk_d[:])
        bvr = const.tile([1, 512], F32, tag="bvr")
        nc.sync.dma_start(bvr[:], bv_d[:])
        mck = const.tile([128, 2048], F32, tag="mck")
        nc.sync.dma_start(mck[:], mk_d[:])
        ones1 = const.tile([1, 128], F32, tag="ones1")
        nc.vector.memset(ones1[:], 1.0)
        ones128 = const.tile([128, 128], F32, tag="ones128")
        nc.vector.memset(ones128[:], 1.0)
        ident = mck[:, 0:128]
        emask = mck[:, 128:2048]           # [128, 1920]

        ps = psp.tile([128, 4096], F32, tag="ps")

        for rep in range(reps):
            # ---- x load: all 4 batches -> [128, 2(ci), 4096] ----
            xt = xp.tile([128, 2, SB], F32, tag="xt")
            nc.sync.dma_start(xt[:], x_d[:].rearrange("ci p b s -> p ci (b s)"))
            # fp8 copy of x for the Q/K path (SWDGE casting DMA)
            xt8 = xp.tile([128, 2, SB], FP8, tag="xt8")
            nc.gpsimd.dma_start(xt8[:],
                                x_d[:].rearrange("ci p b s -> p ci (b s)"))

            # ---- Q/K projections (fp8 DoubleRow), all batches at once ----
            q = qkp.tile([128, 2, SB], FP8, tag="q")
            k = qkp.tile([128, 2, SB], FP8, tag="k")
            for pi, (wt, dst) in enumerate(((wqt, q), (wkt, k))):
                for m in range(2):
                    for c in range(8):
                        cs = slice(512 * c, 512 * c + 512)
                        nc.tensor.matmul(ps[:, cs],
                                         wt[:, :, 128 * m:128 * m + 128],
                                         xt8[:, :, cs], start=True, stop=True,
                                         perf_mode=DR)
                    nc.vector.tensor_scalar(dst[:, m, :], ps[:],
                                            bqk[:, 2 * pi + m:2 * pi + m + 1],
                                            0.0, op0=ALU.add, op1=ALU.max)

            for b in range(NB):
                s0 = b * S

                # ---- V projection -> vT [128(t), 8, 256(v)] ----
                vt = vtp.tile([128, 8, 256], F32, tag="vt")
                for a in range(8):
                    for ci in range(2):
                        nc.tensor.matmul(
                            ps[:, 256 * a:256 * a + 256],
                            xt[:, ci, s0 + 128 * a:s0 + 128 * a + 128],
                            wvt[:, ci, :],
                            start=(a % 2 == 0 and ci == 0), stop=False)
                for h in range(4):
                    nc.tensor.matmul(ps[:, 512 * h:512 * h + 512],
                                     ones1[:], bvr[:],
                                     start=False, stop=True)
                nc.vector.tensor_scalar(
                    vt[:], ps[:, 0:2048].rearrange("p (a v) -> p a v", a=8),
                    0.0, None, op0=ALU.max)

                # ---- scores + exp: 2 phases of 4 t-tiles ----
                pt = ppool.tile([128, 8, S], F32, tag="pt")
                for ph in range(2):
                    for j in range(4):
                        i = 4 * ph + j
                        tc_ = slice(s0 + 128 * i, 128 * i + s0 + 128)
                        for c in range(2):
                            sc = ps[:, 1024 * j + 512 * c:
                                    1024 * j + 512 * c + 512]
                            j0 = 896 - 128 * i + 512 * c
                            fully_masked = j0 <= 384
                            has_mask = j0 < 1024
                            if not fully_masked:
                                qs = slice(s0 + 512 * c, s0 + 512 * c + 512)
                                nc.tensor.matmul(sc, k[:, :, tc_],
                                                 q[:, :, qs], start=True,
                                                 stop=not has_mask,
                                                 perf_mode=DR)
                            if has_mask:
                                nc.tensor.matmul(sc, ident,
                                                 emask[:, j0:j0 + 512],
                                                 start=fully_masked, stop=True)
                    nc.scalar.activation(
                        pt[:, 4 * ph:4 * ph + 4, :],
                        ps[:].rearrange("p (a s) -> p a s", a=4),
                        AF.Exp, scale=0.0625)

                # ---- o / dn ----
                om0 = ps[:, 0:1024]
                om1 = ps[:, 1024:2048]
                dnp = ps[:, 2048:3072]
                for c in range(2):
                    ntile = 4 * c + 4   # causal: chunk c only sees t < 512(c+1)
                    for i in range(ntile):
                        st, sp_ = (i == 0), (i == ntile - 1)
                        pcs = pt[:, i, 512 * c:512 * c + 512]
                        for roff, lh in ((0, vt[:, i, 0:128]),
                                         (1024, vt[:, i, 128:256]),
                                         (2048, ones128[:])):
                            nc.tensor.matmul(
                                ps[:, roff + 512 * c:roff + 512 * c + 512],
                                lh, pcs, start=st, stop=sp_)

                # ---- normalize + store ----
                rec = rp.tile([128, S], F32, tag="rec")
                nc.vector.reciprocal(rec[:], dnp)
                osb = op.tile([128, 2, S], F32, tag="osb")
                nc.vector.tensor_tensor(osb[:, 0, :], om0, rec[:], op=ALU.mult)
                nc.vector.tensor_tensor(osb[:, 1, :], om1, rec[:], op=ALU.mult)
                nc.sync.dma_start(
                    out_d[b][:].rearrange("(m p) s -> p m s", p=128), osb[:])

    nc.finalize()
    return nc


def make_in_maps(x, wq, bq, wk, bk, wv, bv):
    x_r = x.reshape(B, C, S).astype(np.float32)
    pe = _pos_embeddings()
    xm = np.concatenate(
        [x_r, np.broadcast_to(pe[None], (B, 2 * EMB, S))], axis=1)
    # [B, 2, 128, S] -> per-core [2, 128, NB, S] so the on-device DMA can
    # group (b s) into adjacent columns
    xm = xm.reshape(B, 2, 128, S)
    wqt = np.ascontiguousarray(wq.T.astype(np.float32))
    wkt = np.ascontiguousarray(wk.T.astype(np.float32))
    wvt = np.ascontiguousarray(wv.T.astype(np.float32))
    bq = bq.astype(np.float32)
    bk = bk.astype(np.float32)
    bqk = np.ascontiguousarray(
        np.stack([bq[:128], bq[128:], bk[:128], bk[128:]], axis=1))
    bvr = np.ascontiguousarray(np.tile(bv.astype(np.float32), 2)[None, :])
    ident = np.eye(128, dtype=np.float32)
    jj = np.arange(1920)[None, :]
    ppn = np.arange(128)[:, None]
    emask = np.where(jj < 896 + ppn, np.float32(MASKV), np.float32(0.0))
    mconst = np.ascontiguousarray(
        np.concatenate([ident, emask.astype(np.float32)], axis=1))
    common = dict(wqt=wqt, wkt=wkt, wvt=wvt, bqk=bqk, bvr=bvr, mconst=mconst)
    return [dict(xm=np.ascontiguousarray(
                     xm[i * NB:(i + 1) * NB].transpose(1, 2, 0, 3)), **common)
            for i in range(NCORES)]


_NC_CACHE = None


def kernel(x, wq, bq, wk, bk, wv, bv):
    global _NC_CACHE
    if _NC_CACHE is None:
        _NC_CACHE = build()
    nc = _NC_CACHE
    in_maps = make_in_maps(x, wq, bq, wk, bk, wv, bv)
    res = run_bass_kernel_spmd(nc, in_maps, core_ids=list(range(NCORES)))
    out = np.concatenate([res.results[i]["out"] for i in range(NCORES)], axis=0)
    return np.ascontiguousarray(out.reshape(B, 256, L, L).astype(np.float32))


# revision 22
# speedup vs baseline: 3.9109x; 1.0976x over previous
"""Trainium2 Bass kernel v3: causal spatial attention block, all-f32,
minimal-instruction design.

Data-parallel over batch across 8 NeuronCores (4 batches/core, no
collectives). The execution backend charges a large, mostly
size-independent cost per *instruction* (ACT worst, ~250us; MM ~30-50us;
DVE ~30us), so this kernel minimizes instruction count and avoids narrow
dtypes entirely (fp8/bf16 conversions are pure overhead there):
  - Q/K projections: 8 giant matmuls of N=4096 (all 4 batches at once)
    into full-PSUM [128,4096] tiles; one fused bias+relu drain per
    (proj, chan-half) -> 4 DVE instructions.
  - V projection per batch: 16 matmuls produce vT directly in [t, v]
    layout (lhsT = x-tile, rhs = wv), 2 rank-1 bias matmuls, one relu
    drain.
  - scores per batch: 2 phases x 4 t-tiles x (2 k-matmuls N=1024 + 1
    additive-mask matmul via identity-lhsT x shifted-window of one
    [128,1920] mask pattern); exp(score/16) applied by ONE activation
    per phase over the whole [128,4,1024] PSUM; masked entries underflow
    to exact f32 zeros, so o/dn can contract full 1024-wide tiles.
  - o/dn per batch: 24 matmuls (8 t-tiles x {v-half0, v-half1, ones});
    reciprocal + 2 multiplies normalize during the PSUM drain.
All PSUM phases share one [128,4096] tile (8 banks), start=True only on
the first matmul touching each 512-col bank.
"""
import numpy as np
from contextlib import ExitStack

import concourse.bass as bass
import concourse.mybir as mybir
import concourse.tile as tile
from concourse import bacc
from concourse.bass_utils import run_bass_kernel_spmd

F32 = mybir.dt.float32
FP8 = mybir.dt.float8e4
AF = mybir.ActivationFunctionType
ALU = mybir.AluOpType
DR = mybir.MatmulPerfMode.DoubleRow

B, C, L, EMB = 32, 224, 32, 16
S = L * L            # 1024
CIN = 256
NCORES = 8
NB = B // NCORES     # 4 batches per core
SB = NB * S          # 4096 columns: all 4 batches side by side
MASKV = -30000.0     # exp((x-30000)/16) underflows to exact f32 zero


def _pos_embeddings() -> np.ndarray:
    pos = np.arange(L)[:, None].astype(np.float64)
    j = np.arange(EMB)[None, :]
    enc = pos / np.power(10000.0, 2.0 * (j // 2) / EMB)
    enc[0, :] = 0.0
    enc[1:, 0::2] = np.sin(enc[1:, 0::2])
    enc[1:, 1::2] = np.cos(enc[1:, 1::2])
    t = enc.astype(np.float32)
    x = np.tile(t.reshape(1, EMB, L, 1), (1, 1, 1, L))
    y = np.tile(t.reshape(1, EMB, 1, L), (1, 1, L, 1))
    pe = np.concatenate((x, y), axis=1)[0]
    return np.ascontiguousarray(pe.reshape(2 * EMB, S))


def build(reps: int = 1):
    nc = bacc.Bacc("TRN2", target_bir_lowering=False, debug=False,
                   num_devices=NCORES)
    x_d = nc.declare_dram_parameter("xm", [2, 128, NB, S], F32, isOutput=False)
    wq_d = nc.declare_dram_parameter("wqt", [CIN, 256], F32, isOutput=False)
    wk_d = nc.declare_dram_parameter("wkt", [CIN, 256], F32, isOutput=False)
    wv_d = nc.declare_dram_parameter("wvt", [CIN, 256], F32, isOutput=False)
    bqk_d = nc.declare_dram_parameter("bqk", [128, 4], F32, isOutput=False)
    bv_d = nc.declare_dram_parameter("bvr", [1, 512], F32, isOutput=False)
    # mconst: [ identity(128) | E-mask(1920) ]  (E[p][j] = MASKV if j < 896+p)
    mk_d = nc.declare_dram_parameter("mconst", [128, 2048], F32, isOutput=False)
    out_d = nc.declare_dram_parameter("out", [NB, 256, S], F32, isOutput=True)

    with ExitStack() as ctx:
        tc = ctx.enter_context(tile.TileContext(nc))
        const = ctx.enter_context(tc.tile_pool(name="const", bufs=1))
        xp = ctx.enter_context(tc.tile_pool(name="xp", bufs=1))
        qkp = ctx.enter_context(tc.tile_pool(name="qk", bufs=1))
        vtp = ctx.enter_context(tc.tile_pool(name="vt", bufs=2))
        ppool = ctx.enter_context(tc.tile_pool(name="pp", bufs=1))
        op = ctx.enter_context(tc.tile_pool(name="ob", bufs=2))
        rp = ctx.enter_context(tc.tile_pool(name="rp", bufs=2))
        psp = ctx.enter_context(tc.tile_pool(name="ps", bufs=1, space="PSUM"))

        # ---- constants ----
        # SWDGE casting DMAs: f32 DRAM -> fp8 SBUF, bit-exact RNE
        wqt = const.tile([128, 2, 256], FP8, tag="wqt")
        nc.gpsimd.dma_start(wqt[:],
                            wq_d[:].rearrange("(ci p) m -> p ci m", p=128))
        wkt = const.tile([128, 2, 256], FP8, tag="wkt")
        nc.gpsimd.dma_start(wkt[:],
                            wk_d[:].rearrange("(ci p) m -> p ci m", p=128))
        wvt = const.tile([128, 2, 256], F32, tag="wvt")
        nc.sync.dma_start(wvt[:], wv_d[:].rearrange("(ci p) m -> p ci m", p=128))
        bqk = const.tile([128, 4], F32, tag="bqk")
        nc.sync.dma_start(bqk[:], bqk_d[:])
        bvr = const.tile([1, 512], F32, tag="bvr")
        nc.sync.dma_start(bvr[:], bv_d[:])
        mck = const.tile([128, 2048], F32, tag="mck")
        nc.sync.dma_start(mck[:], mk_d[:])
        ones1 = const.tile([1, 128], F32, tag="ones1")
        nc.vector.memset(ones1[:], 1.0)
        ones8 = const.tile([128, 2, 128], FP8, tag="ones8")
        nc.vector.memset(ones8[:], 1.0)
        ident = mck[:, 0:128]
        emask = mck[:, 128:2048]           # [128, 1920]

        ps = psp.tile([128, 4096], F32, tag="ps")

        for rep in range(reps):
            # ---- x load: all 4 batches -> [128, 2(ci), 4096] ----
            xt = xp.tile([128, 2, SB], F32, tag="xt")
            nc.sync.dma_start(xt[:], x_d[:].rearrange("ci p b s -> p ci (b s)"))
            # fp8 copy of x for the Q/K path (SWDGE casting DMA)
            xt8 = xp.tile([128, 2, SB], FP8, tag="xt8")
            nc.gpsimd.dma_start(xt8[:],
                                x_d[:].rearrange("ci p b s -> p ci (b s)"))

            # ---- Q/K projections (fp8 DoubleRow), all batches at once ----
            q = qkp.tile([128, 2, SB], FP8, tag="q")
            k = qkp.tile([128, 2, SB], FP8, tag="k")
            for pi, (wt, dst) in enumerate(((wqt, q), (wkt, k))):
                for m in range(2):
                    for c in range(8):
                        cs = slice(512 * c, 512 * c + 512)
                        nc.tensor.matmul(ps[:, cs],
                                         wt[:, :, 128 * m:128 * m + 128],
                                         xt8[:, :, cs], start=True, stop=True,
                                         perf_mode=DR)
                    nc.vector.tensor_scalar(dst[:, m, :], ps[:],
                                            bqk[:, 2 * pi + m:2 * pi + m + 1],
                                            0.0, op0=ALU.add, op1=ALU.max)

            for b in range(NB):
                s0 = b * S

                # ---- V projection -> vT [128(t), 8, 256(v)] ----
                vt = vtp.tile([128, 8, 256], F32, tag="vt")
                for a in range(8):
                    for ci in range(2):
                        nc.tensor.matmul(
                            ps[:, 256 * a:256 * a + 256],
                            xt[:, ci, s0 + 128 * a:s0 + 128 * a + 128],
                            wvt[:, ci, :],
                            start=(a % 2 == 0 and ci == 0), stop=False)
                for h in range(4):
                    nc.tensor.matmul(ps[:, 512 * h:512 * h + 512],
                                     ones1[:], bvr[:],
                                     start=False, stop=True)
                nc.vector.tensor_scalar(
                    vt[:], ps[:, 0:2048].rearrange("p (a v) -> p a v", a=8),
                    0.0, None, op0=ALU.max)
                # fp8 copy for the DoubleRow o/dn matmuls (Pool engine)
                vt8 = vtp.tile([128, 8, 256], FP8, tag="vt8")
                nc.gpsimd.tensor_copy(vt8[:], vt[:])

                # ---- scores + exp: 2 phases of 4 t-tiles ----
                # Fully-masked chunks (i>=4, s-chunk 0) are skipped outright:
                # the exp there reads this batch's phase-0 leftovers (finite,
                # saturates in fp8) and o/dn never contracts that region.
                pt = ppool.tile([128, 8, S], FP8, tag="pt")
                for ph in range(2):
                    for j in range(4):
                        i = 4 * ph + j
                        tc_ = slice(s0 + 128 * i, 128 * i + s0 + 128)
                        for c in range(2):
                            sc = ps[:, 1024 * j + 512 * c:
                                    1024 * j + 512 * c + 512]
                            j0 = 896 - 128 * i + 512 * c
                            if j0 <= 384:
                                continue
                            has_mask = j0 < 1024
                            qs = slice(s0 + 512 * c, s0 + 512 * c + 512)
                            nc.tensor.matmul(sc, k[:, :, tc_],
                                             q[:, :, qs], start=True,
                                             stop=not has_mask,
                                             perf_mode=DR)
                            if has_mask:
                                nc.tensor.matmul(sc, ident,
                                                 emask[:, j0:j0 + 512],
                                                 start=False, stop=True)
                    nc.scalar.activation(
                        pt[:, 4 * ph:4 * ph + 4, :],
                        ps[:].rearrange("p (a s) -> p a s", a=4),
                        AF.Exp, scale=0.0625)

                # ---- o / dn ----
                om0 = ps[:, 0:1024]
                om1 = ps[:, 1024:2048]
                dnp = ps[:, 2048:3072]
                for c in range(2):
                    npair = 2 * c + 2   # causal: chunk c only sees t < 512(c+1)
                    for a in range(npair):
                        st, sp_ = (a == 0), (a == npair - 1)
                        pcs = pt[:, 2 * a:2 * a + 2, 512 * c:512 * c + 512]
                        for roff, lh in ((0, vt8[:, 2 * a:2 * a + 2, 0:128]),
                                         (1024, vt8[:, 2 * a:2 * a + 2,
                                                     128:256]),
                                         (2048, ones8[:])):
                            nc.tensor.matmul(
                                ps[:, roff + 512 * c:roff + 512 * c + 512],
                                lh, pcs, start=st, stop=sp_, perf_mode=DR)

                # ---- normalize + store ----
                rec = rp.tile([128, S], F32, tag="rec")
                nc.vector.reciprocal(rec[:], dnp)
                osb = op.tile([128, 2, S], F32, tag="osb")
                nc.vector.tensor_tensor(osb[:, 0, :], om0, rec[:], op=ALU.mult)
                nc.vector.tensor_tensor(osb[:, 1, :], om1, rec[:], op=ALU.mult)
                nc.sync.dma_start(
                    out_d[b][:].rearrange("(m p) s -> p m s", p=128), osb[:])

    nc.finalize()
    return nc


def make_in_maps(x, wq, bq, wk, bk, wv, bv):
    x_r = x.reshape(B, C, S).astype(np.float32)
    pe = _pos_embeddings()
    xm = np.concatenate(
        [x_r, np.broadcast_to(pe[None], (B, 2 * EMB, S))], axis=1)
    # [B, 2, 128, S] -> per-core [2, 128, NB, S] so the on-device DMA can
    # group (b s) into adjacent columns
    xm = xm.reshape(B, 2, 128, S)
    wqt = np.ascontiguousarray(wq.T.astype(np.float32))
    wkt = np.ascontiguousarray(wk.T.astype(np.float32))
    wvt = np.ascontiguousarray(wv.T.astype(np.float32))
    bq = bq.astype(np.float32)
    bk = bk.astype(np.float32)
    bqk = np.ascontiguousarray(
        np.stack([bq[:128], bq[128:], bk[:128], bk[128:]], axis=1))
    bvr = np.ascontiguousarray(np.tile(bv.astype(np.float32), 2)[None, :])
    ident = np.eye(128, dtype=np.float32)
    jj = np.arange(1920)[None, :]
    ppn = np.arange(128)[:, None]
    emask = np.where(jj < 896 + ppn, np.float32(MASKV), np.float32(0.0))
    mconst = np.ascontiguousarray(
        np.concatenate([ident, emask.astype(np.float32)], axis=1))
    common = dict(wqt=wqt, wkt=wkt, wvt=wvt, bqk=bqk, bvr=bvr, mconst=mconst)
    return [dict(xm=np.ascontiguousarray(
                     xm[i * NB:(i + 1) * NB].transpose(1, 2, 0, 3)), **common)
            for i in range(NCORES)]


_NC_CACHE = None


def kernel(x, wq, bq, wk, bk, wv, bv):
    global _NC_CACHE
    if _NC_CACHE is None:
        _NC_CACHE = build()
    nc = _NC_CACHE
    in_maps = make_in_maps(x, wq, bq, wk, bk, wv, bv)
    res = run_bass_kernel_spmd(nc, in_maps, core_ids=list(range(NCORES)))
    out = np.concatenate([res.results[i]["out"] for i in range(NCORES)], axis=0)
    return np.ascontiguousarray(out.reshape(B, 256, L, L).astype(np.float32))
